# revision 1
# baseline (speedup 1.0000x reference)
"""Class-based decoder (MoE-style routing) on 8 trn2 NeuronCores.

Strategy: expert-parallel. Classes are padded 250->256 and split 32 per core.
On the host, tokens are grouped by class into capacity-padded slots (C tokens
per class slot, C in {32,64,128}); class slots that overflow C spill into
extra slots holding a duplicate of the class weights.  Each core receives:
  - xT   [128, n_mt*KCH*128]   its padded tokens, pre-transposed k-major
  - wcT  [128, KCH*NCLS_P]     the (replicated) class-decoder weights, k-major
  - wwT  [n_grp, 128, GRP*KCH*CHUNK]  its word-decoder shard, k-major, grouped
and computes, for every 128-token tile, the class logits (x @ Wc.T) and the
per-class word logits (x_c @ Ww[c].T) as PE matmuls accumulating K=512 over
4 PSUM chunks.  Class slots of a tile are col-tiled into one PSUM tile.
Biases (zero in practice, but handled for correctness) are added on the host
during the final unpermute.

Precision modes:
  f32  : exact fp32 matmuls (2-pass LOW/HIGH on PE; bit-exact, slowest)
  f32r : fp32 data, single-pass PE (TF32-like rounding). Classes are paired
         into N=400 matmuls and NCLS padded to 256 so the free dim is >=256,
         where f32r streams at full rate.
  bf16 : weights/activations cast to bf16 (halves the big W DMA)
"""

import numpy as np
from contextlib import ExitStack

import concourse.bass as bass
import concourse.bacc as bacc
import concourse.tile as tile
import concourse.mybir as mybir
from concourse.bass_utils import run_bass_kernel_spmd

NHID = 512
NCLS = 250
CHUNK = 200
NCORES = 8
KCH = NHID // 128          # 4 contraction chunks of 128
NCLS_PAD = 256             # classes padded so each core owns an equal shard
CPC = NCLS_PAD // NCORES   # classes per core
NCOL = NCLS + CHUNK        # 450 output columns
F32 = mybir.dt.float32

MODE = "bf16"              # default precision mode; see module docstring

LAST_RESULT = None         # BassKernelResults of the most recent device run
_program_cache = {}

_MM_DT = {"f32": mybir.dt.float32, "f32r": mybir.dt.float32r,
          "bf16": mybir.dt.bfloat16}
_NP_DT = {"f32": np.float32, "f32r": np.float32, "bf16": None}  # bf16 set below
try:
    import ml_dtypes
    _NP_DT["bf16"] = ml_dtypes.bfloat16
except ImportError:
    pass


def _build_program(C, slots, mode):
    """One SPMD program: slots class-slots of C tokens each, per core.

    f32 uses the "coltile" scheme: per class slot, an M=C matmul col-tiled
    into a shared PSUM tile (exact 2-pass fp32).
    f32r/bf16 use the "block" scheme: every matmul is M=128 (all slots of an
    m-tile), and the word logits come as per_mt//2 halves of N=2*CHUNK whose
    off-diagonal class blocks are discarded by the PSUM->SBUF copies.  This
    keeps N>=256 (full-rate f32r) and NumWeights=128 (FWL weight loads).
    """
    n_mt = (slots * C) // 128  # 128-token m-tiles
    npad = slots * C
    per_mt = 128 // C          # class slots per m-tile
    block = mode in ("f32r", "bf16")
    # class slots per pw matmul and word-columns per pw matmul
    gs = 2 if (block and per_mt >= 2) else 1
    gw = gs * CHUNK            # 400 paired / 200 single
    n_half = per_mt // gs      # pw matmul groups per m-tile
    ncls_p = 256 if block else NCLS  # N>=256 keeps f32r at full rate
    # C=16 diag copies would need 16-partition bases (illegal); store each
    # 32-row band's full pair block instead and let the host pick the diagonal
    wide = block and C == 16
    ocol = NCLS + (gw if wide else CHUNK)
    dt = _MM_DT[mode]

    nc = bacc.Bacc("TRN2", target_bir_lowering=False, debug=False,
                   num_devices=NCORES)
    xT = nc.dram_tensor("xT", [128, n_mt * KCH * 128], dt, kind="ExternalInput")
    wcT = nc.dram_tensor("wcT", [128, KCH * ncls_p], dt, kind="ExternalInput")
    # W groups: one DMA per m-tile worth of class slots
    wwT = nc.dram_tensor("wwT", [n_mt, 128, per_mt * KCH * CHUNK], dt,
                         kind="ExternalInput")
    out = nc.dram_tensor("out", [npad, ocol], F32, kind="ExternalOutput")

    with tile.TileContext(nc) as tc, ExitStack() as ctx:
        xpool = ctx.enter_context(tc.tile_pool(name="x", bufs=1))
        wcpool = ctx.enter_context(tc.tile_pool(name="wc", bufs=1))
        wpool = ctx.enter_context(tc.tile_pool(name="w", bufs=12))
        opool = ctx.enter_context(tc.tile_pool(name="o", bufs=8))
        pcp = ctx.enter_context(
            tc.tile_pool(name="pc", bufs=2, space=bass.MemorySpace.PSUM))
        pwp = ctx.enter_context(
            tc.tile_pool(name="pw", bufs=6, space=bass.MemorySpace.PSUM))

        # three independent DMA streams so nothing blocks the W firehose:
        #   sync (SP HWDGE): only the big W chunks, back to back
        #   scalar (ACT HWDGE): wc + per-m-tile x loads
        #   gpsimd (SWDGE): output stores
        wc_sb = wcpool.tile([128, KCH * ncls_p], dt)
        nc.scalar.dma_start(wc_sb[:], wcT[:])
        x_sb = xpool.tile([128, n_mt * KCH * 128], dt)

        wchunk = KCH * gw  # free-dim elems per W DMA (one pw matmul group)
        for m in range(n_mt):
            # x columns for this m-tile: [(m*KCH+j)*128 + t]
            nc.scalar.dma_start(x_sb[:, m * KCH * 128:(m + 1) * KCH * 128],
                                xT[:, m * KCH * 128:(m + 1) * KCH * 128])

            def xcol(j, lo, hi):
                base = (m * KCH + j) * 128
                return x_sb[:, base + lo:base + hi]

            # class logits for these 128 tokens
            pc_ps = pcp.tile([128, ncls_p], F32)
            for j in range(KCH):
                nc.tensor.matmul(
                    pc_ps[:, :],
                    xcol(j, 0, 128),
                    wc_sb[:, j * ncls_p:(j + 1) * ncls_p],
                    start=(j == 0), stop=(j == KCH - 1),
                )

            o_sb = opool.tile([128, ocol], F32)
            nc.vector.tensor_copy(o_sb[:, :NCLS], pc_ps[:, :NCLS])

            if block:
                # word logits: per half, one M=128 matmul of N=gw covering
                # gs classes; only each slot's own class block is kept
                for h in range(n_half):
                    w_sb = wpool.tile([128, wchunk], dt, tag="w")
                    weng = nc.sync if (m * n_half + h) % 2 == 0 else nc.scalar
                    weng.dma_start(
                        w_sb[:], wwT[m][:, h * wchunk:(h + 1) * wchunk])
                    pw_ps = pwp.tile([128, gw], F32, tag="pw")
                    for j in range(KCH):
                        nc.tensor.matmul(
                            pw_ps[:, :],
                            xcol(j, 0, 128),
                            w_sb[:, j * gw:(j + 1) * gw],
                            start=(j == 0), stop=(j == KCH - 1),
                        )
                    if wide:
                        b = gs * C  # 32-row band of this pair
                        nc.vector.tensor_copy(
                            o_sb[h * b:(h + 1) * b, NCLS:],
                            pw_ps[h * b:(h + 1) * b, :])
                    else:
                        for a in range(gs):
                            q = h * gs + a  # slot in m-tile
                            nc.vector.tensor_copy(
                                o_sb[q * C:(q + 1) * C, NCLS:],
                                pw_ps[q * C:(q + 1) * C,
                                      a * CHUNK:(a + 1) * CHUNK])
            else:
                # exact f32: per-slot M=C matmuls col-tiled into one tile
                w_sb = wpool.tile([128, per_mt * KCH * CHUNK], dt, tag="w")
                nc.sync.dma_start(w_sb[:], wwT[m])
                pw_ps = pwp.tile([128, CHUNK], F32, tag="pw")
                for q in range(per_mt):
                    for j in range(KCH):
                        nc.tensor.matmul(
                            pw_ps[q * C:(q + 1) * C, :],
                            xcol(j, q * C, (q + 1) * C),
                            w_sb[:, (q * KCH + j) * CHUNK:
                                 (q * KCH + j + 1) * CHUNK],
                            start=(j == 0), stop=(j == KCH - 1),
                            tile_position=(0, q * C),
                        )
                nc.vector.tensor_copy(o_sb[:, NCLS:], pw_ps[:])

            nc.gpsimd.dma_start(out[m * 128:(m + 1) * 128, :], o_sb[:])

    nc.compile()
    return nc


def _route(cls, mode):
    """Group tokens by class into capacity-padded slots: one slot per class,
    C tokens of capacity.  The (rare) tokens beyond a class's capacity are
    returned as `overflow` and evaluated directly on the host in numpy.

    Returns (C, slots, tok_idx [NCORES, slots*C] int64 token id or -1,
    slot_cls [NCORES, slots] class id per slot, overflow token-id array).
    """
    counts = np.bincount(cls, minlength=NCLS_PAD)
    # coltile (exact f32) needs C to be a multiple of 32 for PSUM col tiling
    cands = (16, 32, 64, 128) if mode in ("f32r", "bf16") else (32, 64, 128)
    C = cands[-1]
    for c in cands:
        if int(np.maximum(counts - c, 0).sum()) <= 32:
            C = c
            break

    order = np.argsort(cls, kind="stable")
    starts = np.zeros(NCLS_PAD + 1, np.int64)
    starts[1:] = np.cumsum(counts)

    slots = CPC  # one slot per class owned by the core
    tok_idx = np.full((NCORES, slots * C), -1, np.int64)
    slot_cls = np.full((NCORES, slots), -1, np.int64)
    overflow = []
    for k in range(NCORES):
        for s in range(slots):
            c = k * CPC + s
            lo, cnt = int(starts[c]), int(counts[c])
            n = min(C, cnt)
            slot_cls[k, s] = c
            if n > 0:
                tok_idx[k, s * C:s * C + n] = order[lo:lo + n]
            if cnt > C:
                overflow.append(order[lo + C:lo + cnt])
    overflow = (np.concatenate(overflow) if overflow
                else np.zeros((0,), np.int64))
    return C, slots, tok_idx, slot_cls, overflow


def kernel(x, Wc, bc, Ww, bw, cls_idx, _trace=False, _trace_cores=None,
           _mode=None):
    global LAST_RESULT
    mode = _mode or MODE
    ndt = _NP_DT[mode]
    if ndt is None:
        mode = "f32"
        ndt = np.float32

    x = np.ascontiguousarray(np.asarray(x, np.float32))
    Wc = np.ascontiguousarray(np.asarray(Wc, np.float32))
    bc = np.asarray(bc, np.float32)
    Ww = np.ascontiguousarray(np.asarray(Ww, np.float32))
    bw = np.asarray(bw, np.float32)
    cls = np.asarray(cls_idx).astype(np.int64).ravel()
    N = cls.shape[0]

    C, slots, tok_idx, slot_cls, overflow = _route(cls, mode)
    npad = slots * C
    n_mt = npad // 128
    per_mt = 128 // C
    block = mode in ("f32r", "bf16")
    gs = 2 if (block and per_mt >= 2) else 1
    ncls_p = 256 if block else NCLS

    key = (C, slots, mode)
    if key not in _program_cache:
        _program_cache[key] = _build_program(C, slots, mode)
    nc = _program_cache[key]

    # wcT [128, KCH*ncls_p]: wcT[p, j*ncls_p+c] = Wc[c, j*128+p]  (replicated)
    Wc_p = Wc if ncls_p == NCLS else np.concatenate(
        [Wc, np.zeros((ncls_p - NCLS, NHID), np.float32)], 0)
    wcT = np.ascontiguousarray(
        Wc_p.reshape(ncls_p, KCH, 128).transpose(2, 1, 0)
            .reshape(128, KCH * ncls_p).astype(ndt))

    Ww_pad = np.zeros((NCLS_PAD, CHUNK, NHID), np.float32)
    Ww_pad[:NCLS] = Ww

    in_maps = []
    for k in range(NCORES):
        # per-slot k-major weights: tmp[s, j, p, w] = Ww[cls_s, w, j*128+p]
        wsel = Ww_pad[np.maximum(slot_cls[k], 0)]
        wsel[slot_cls[k] < 0] = 0.0
        tmp = wsel.reshape(slots, CHUNK, KCH, 128).transpose(0, 2, 3, 1)
        if gs == 2:
            # group = m-tile (per_mt slots); within: pair r, then j, then
            # the two slots' CHUNK columns side by side
            tmp = tmp.reshape(n_mt, per_mt // 2, 2, KCH, 128, CHUNK)
            tmp = tmp.transpose(0, 4, 1, 3, 2, 5)  # [n_mt,128,pair,j,2,CHUNK]
        else:
            tmp = tmp.reshape(n_mt, per_mt, KCH, 128, CHUNK)
            tmp = tmp.transpose(0, 3, 1, 2, 4)     # [n_mt,128,q,j,CHUNK]
        wwT = np.ascontiguousarray(
            tmp.reshape(n_mt, 128, per_mt * KCH * CHUNK).astype(ndt))

        ti = tok_idx[k]
        xk = x[np.maximum(ti, 0)]
        xk[ti < 0] = 0.0
        # xT[p, (m*KCH+j)*128 + t] = xk[m*128+t, j*128+p]
        xT = np.ascontiguousarray(
            xk.reshape(n_mt, 128, KCH, 128).transpose(3, 0, 2, 1)
              .reshape(128, n_mt * KCH * 128).astype(ndt))
        in_maps.append({"xT": xT, "wcT": wcT, "wwT": wwT})

    LAST_RESULT = run_bass_kernel_spmd(
        nc, in_maps, list(range(NCORES)), trace=_trace,
        trace_cores=(_trace_cores if _trace else None))

    wide = block and C == 16
    out = np.zeros((N, NCOL), np.float32)
    if wide:
        # row r of a core's output holds its pair's full 2*CHUNK block;
        # slot parity selects which CHUNK half is this row's class
        a_row = (np.arange(npad) // C) % 2
    for k in range(NCORES):
        ok = np.asarray(LAST_RESULT.results[k]["out"], np.float32)
        if wide:
            words = np.where((a_row == 0)[:, None],
                             ok[:, NCLS:NCLS + CHUNK],
                             ok[:, NCLS + CHUNK:NCLS + 2 * CHUNK])
            ok = np.concatenate([ok[:, :NCLS], words], 1)
        valid = tok_idx[k] >= 0
        out[tok_idx[k][valid]] = ok[valid]

    if overflow.size:
        # rare capacity-overflow tokens: evaluate directly on the host
        xo = x[overflow]                                   # [no, NHID]
        out[overflow, :NCLS] = xo @ Wc.T
        co = cls[overflow]
        out[overflow, NCLS:] = np.einsum(
            "nkh,nh->nk", Ww[co], xo, optimize=True)

    out[:, :NCLS] += bc
    out[:, NCLS:] += bw[cls]
    return out



# revision 4
# speedup vs baseline: 1.0684x; 1.0684x over previous
"""Class-based decoder (MoE-style routing) on 8 trn2 NeuronCores.

Strategy: expert-parallel. Classes are padded 250->256 and split 32 per core.
On the host, tokens are grouped by class into capacity-padded slots (C tokens
per class slot); the rare tokens beyond a class's capacity are evaluated on
the host.  Each core receives its padded tokens pre-transposed k-major (xT,
bf16), the replicated class-decoder weights (wcT, bf16), and its word-decoder
shard (wwT) quantized to float8_e3m4 and pre-scaled by 64 (exact power of two,
divided back out on the host).  Per 128-token m-tile the PE computes class
logits (x @ Wc.T) and, per pair of classes, word logits (x @ [W_a|W_b].T) as
K=512 accumulations with x as the stationary operand and the weights as the
moving operand, so every weight element streams through the PE exactly once.
A row-parity mask selects each token's own class block out of the paired
word logits on-device (DVE), and the [128, 450] output tile is stored bf16.

Why fp8 for Ww only: the 6.5 MB/core word-weight stream is the memory-bound
bottleneck; e3m4 (4 mantissa bits) at scale 64 keeps all values normal and
contributes ~0.8% relative error (measured offline), well under the 2e-2
gate, while x (stationary, shared with the class matmul) stays bf16.
"""

import numpy as np
from contextlib import ExitStack

import concourse.bass as bass
import concourse.bacc as bacc
import concourse.tile as tile
import concourse.mybir as mybir
from concourse.bass_utils import run_bass_kernel_spmd

NHID = 512
NCLS = 250
CHUNK = 200
NCORES = 8
KCH = NHID // 128          # 4 contraction chunks of 128
NCLS_PAD = 256             # classes padded so each core owns an equal shard
CPC = NCLS_PAD // NCORES   # classes per core
NCOL = NCLS + CHUNK        # 450 output columns
F32 = mybir.dt.float32
BF16 = mybir.dt.bfloat16
F8 = mybir.dt.float8e3    # e3m4
WSCALE = 64.0              # Ww pre-scale (power of two; divided out on host)

LAST_RESULT = None         # BassKernelResults of the most recent device run
_program_cache = {}

try:
    import ml_dtypes
    _BF16_NP = ml_dtypes.bfloat16
    _F8_NP = ml_dtypes.float8_e3m4
except ImportError:  # pragma: no cover - ml_dtypes ships with jax
    _BF16_NP = None
    _F8_NP = None


def _build_program(C, slots):
    """One SPMD program: slots class-slots of C tokens each, per core.

    Word logits come in pairs of classes (gs=2) so the moving operand is
    N=400 wide; the off-diagonal class block of each 2C-row band is dropped
    by a masked DVE blend.  For C=128 (one class per m-tile) there is no
    pairing and the PSUM tile is copied directly.
    """
    n_mt = (slots * C) // 128  # 128-token m-tiles
    npad = slots * C
    per_mt = 128 // C          # class slots per m-tile
    gs = 2 if per_mt >= 2 else 1
    gw = gs * CHUNK            # moving-operand width per word matmul
    n_half = per_mt // gs      # word matmul groups per m-tile
    hpd = min(2, n_half)       # halves per W DMA chunk
    n_wdma = n_half // hpd     # W DMA chunks per m-tile
    wchunk = hpd * KCH * gw    # free-dim elems per W DMA chunk
    ncls_p = 256

    nc = bacc.Bacc("TRN2", target_bir_lowering=False, debug=False,
                   num_devices=NCORES)
    xT = nc.dram_tensor("xT", [128, n_mt * KCH * 128], BF16,
                        kind="ExternalInput")
    wcT = nc.dram_tensor("wcT", [128, KCH * ncls_p], BF16,
                         kind="ExternalInput")
    wwT = nc.dram_tensor("wwT", [n_mt, 128, n_half * KCH * gw], F8,
                         kind="ExternalInput")
    msk = nc.dram_tensor("msk", [128, 2], F32, kind="ExternalInput")
    out = nc.dram_tensor("out", [npad, NCOL], BF16, kind="ExternalOutput")

    with tile.TileContext(nc) as tc, ExitStack() as ctx:
        xpool = ctx.enter_context(tc.tile_pool(name="x", bufs=1))
        wcpool = ctx.enter_context(tc.tile_pool(name="wc", bufs=1))
        mpool = ctx.enter_context(tc.tile_pool(name="m", bufs=1))
        wpool = ctx.enter_context(tc.tile_pool(name="w", bufs=6))
        opool = ctx.enter_context(tc.tile_pool(name="o", bufs=4))
        dpool = ctx.enter_context(tc.tile_pool(name="d", bufs=4))
        wmpool = ctx.enter_context(tc.tile_pool(name="wm", bufs=1))
        pcp = ctx.enter_context(
            tc.tile_pool(name="pc", bufs=2, space=bass.MemorySpace.PSUM))
        pwp = ctx.enter_context(
            tc.tile_pool(name="pw", bufs=4, space=bass.MemorySpace.PSUM))
        pmp = ctx.enter_context(
            tc.tile_pool(name="pm", bufs=1, space=bass.MemorySpace.PSUM))

        # PE warm-up: HAM unthrottles after ~3.4us of sustained activity, and
        # the DMA pipeline takes ~2us to deliver the first real operands.
        # Burn that dead window with dummy matmuls so real ones start warm.
        warm_sb = wmpool.tile([128, 64], BF16)
        nc.vector.memset(warm_sb[:], 0.0)
        warm_ps = pmp.tile([64, 64], F32)
        for _ in range(18):
            nc.tensor.matmul(warm_ps[:, :], warm_sb[:, :], warm_sb[:, :],
                             start=True, stop=True)

        # W firehose first on the sync HWDGE queue, in consumption order.
        w_sbs = []
        for m in range(n_mt):
            row = []
            for q in range(n_wdma):
                w_sb = wpool.tile([128, wchunk], F8, tag="w")
                nc.sync.dma_start(
                    w_sb[:], wwT[m][:, q * wchunk:(q + 1) * wchunk])
                row.append(w_sb)
            w_sbs.append(row)

        # Small inputs ride the scalar HWDGE queue in parallel.
        x_sb = xpool.tile([128, n_mt * KCH * 128], BF16)
        nc.scalar.dma_start(x_sb[:], xT[:])
        wc_sb = wcpool.tile([128, KCH * ncls_p], BF16)
        nc.scalar.dma_start(wc_sb[:], wcT[:])
        msk_sb = mpool.tile([128, 2], F32)
        nc.scalar.dma_start(msk_sb[:], msk[:])

        for m in range(n_mt):
            def xcol(j):
                base = (m * KCH + j) * 128
                return x_sb[:, base:base + 128]

            # class logits for these 128 tokens
            pc_ps = pcp.tile([128, ncls_p], F32)
            for j in range(KCH):
                nc.tensor.matmul(
                    pc_ps[:, :],
                    xcol(j),
                    wc_sb[:, j * ncls_p:(j + 1) * ncls_p],
                    start=(j == 0), stop=(j == KCH - 1),
                )
            o_sb = opool.tile([128, NCOL], BF16)
            nc.vector.tensor_copy(o_sb[:, :NCLS], pc_ps[:, :NCLS])

            # word logits: per half, one M=128 matmul of N=gw covering gs
            # classes; each 2C-row band keeps only its own class block
            d_sb = dpool.tile([128, CHUNK], F32, tag="d")
            for q in range(n_wdma):
                w_sb = w_sbs[m][q]
                for hh in range(hpd):
                    h = q * hpd + hh
                    pw_ps = pwp.tile([128, gw], F32, tag="pw")
                    for j in range(KCH):
                        nc.tensor.matmul(
                            pw_ps[:, :],
                            xcol(j),
                            w_sb[:, (hh * KCH + j) * gw:
                                 (hh * KCH + j + 1) * gw],
                            start=(j == 0), stop=(j == KCH - 1),
                        )
                    if gs == 1:
                        nc.vector.tensor_copy(o_sb[:, NCLS:], pw_ps[:, :])
                        continue
                    b0, b1 = h * gs * C, (h + 1) * gs * C
                    lo = pw_ps[b0:b1, 0:CHUNK]
                    hi = pw_ps[b0:b1, CHUNK:2 * CHUNK]
                    # o = lo*m + hi*(1-m), m=1 on the band's first C rows
                    nc.vector.tensor_scalar(
                        d_sb[b0:b1, :], lo, msk_sb[b0:b1, 0:1], None,
                        mybir.AluOpType.mult)
                    nc.vector.scalar_tensor_tensor(
                        o_sb[b0:b1, NCLS:], hi, msk_sb[b0:b1, 1:2],
                        d_sb[b0:b1, :],
                        mybir.AluOpType.mult, mybir.AluOpType.add)

            nc.scalar.dma_start(out[m * 128:(m + 1) * 128, :], o_sb[:])

    nc.compile()
    return nc


def _route(cls):
    """Group tokens by class into capacity-padded slots: one slot per class,
    C tokens of capacity.  The (rare) tokens beyond a class's capacity are
    returned as `overflow` and evaluated directly on the host in numpy.

    Returns (C, slots, tok_idx [NCORES, slots*C] int64 token id or -1,
    slot_cls [NCORES, slots] class id per slot, overflow token-id array).
    """
    counts = np.bincount(cls, minlength=NCLS_PAD)
    cands = (16, 32, 64, 128)
    C = cands[-1]
    for c in cands:
        if int(np.maximum(counts - c, 0).sum()) <= 32:
            C = c
            break

    order = np.argsort(cls, kind="stable")
    starts = np.zeros(NCLS_PAD + 1, np.int64)
    starts[1:] = np.cumsum(counts)

    slots = CPC  # one slot per class owned by the core
    tok_idx = np.full((NCORES, slots * C), -1, np.int64)
    slot_cls = np.full((NCORES, slots), -1, np.int64)
    overflow = []
    for k in range(NCORES):
        for s in range(slots):
            c = k * CPC + s
            lo, cnt = int(starts[c]), int(counts[c])
            n = min(C, cnt)
            slot_cls[k, s] = c
            if n > 0:
                tok_idx[k, s * C:s * C + n] = order[lo:lo + n]
            if cnt > C:
                overflow.append(order[lo + C:lo + cnt])
    overflow = (np.concatenate(overflow) if overflow
                else np.zeros((0,), np.int64))
    return C, slots, tok_idx, slot_cls, overflow


def kernel(x, Wc, bc, Ww, bw, cls_idx, _trace=False, _trace_cores=None):
    global LAST_RESULT

    x = np.ascontiguousarray(np.asarray(x, np.float32))
    Wc = np.ascontiguousarray(np.asarray(Wc, np.float32))
    bc = np.asarray(bc, np.float32)
    Ww = np.ascontiguousarray(np.asarray(Ww, np.float32))
    bw = np.asarray(bw, np.float32)
    cls = np.asarray(cls_idx).astype(np.int64).ravel()
    N = cls.shape[0]

    C, slots, tok_idx, slot_cls, overflow = _route(cls)
    npad = slots * C
    n_mt = npad // 128
    per_mt = 128 // C
    gs = 2 if per_mt >= 2 else 1
    ncls_p = 256

    key = (C, slots)
    if key not in _program_cache:
        _program_cache[key] = _build_program(C, slots)
    nc = _program_cache[key]

    # wcT [128, KCH*ncls_p]: wcT[p, j*ncls_p+c] = Wc[c, j*128+p]  (replicated)
    Wc_p = np.concatenate(
        [Wc, np.zeros((ncls_p - NCLS, NHID), np.float32)], 0)
    wcT = np.ascontiguousarray(
        Wc_p.reshape(ncls_p, KCH, 128).transpose(2, 1, 0)
            .reshape(128, KCH * ncls_p).astype(_BF16_NP))

    # row-parity mask: 1 on the first C rows of every 2C-row band
    mrow = ((np.arange(128) % (2 * C)) < C).astype(np.float32)
    msk = np.ascontiguousarray(
        np.stack([mrow, 1.0 - mrow], axis=1))

    Ww_pad = np.zeros((NCLS_PAD, CHUNK, NHID), np.float32)
    Ww_pad[:NCLS] = Ww

    in_maps = []
    for k in range(NCORES):
        # per-slot k-major weights: tmp[s, j, p, w] = Ww[cls_s, w, j*128+p]
        wsel = Ww_pad[np.maximum(slot_cls[k], 0)]
        wsel[slot_cls[k] < 0] = 0.0
        tmp = wsel.reshape(slots, CHUNK, KCH, 128).transpose(0, 2, 3, 1)
        if gs == 2:
            # group = m-tile (per_mt slots); within: pair h, then j, then
            # the two slots' CHUNK columns side by side
            tmp = tmp.reshape(n_mt, per_mt // 2, 2, KCH, 128, CHUNK)
            tmp = tmp.transpose(0, 4, 1, 3, 2, 5)  # [n_mt,128,pair,j,2,CHUNK]
        else:
            tmp = tmp.reshape(n_mt, per_mt, KCH, 128, CHUNK)
            tmp = tmp.transpose(0, 3, 1, 2, 4)     # [n_mt,128,q,j,CHUNK]
        wwT = np.ascontiguousarray(
            (tmp.reshape(n_mt, 128, per_mt * KCH * CHUNK) * WSCALE)
            .astype(_F8_NP))

        ti = tok_idx[k]
        xk = x[np.maximum(ti, 0)]
        xk[ti < 0] = 0.0
        # xT[p, (m*KCH+j)*128 + t] = xk[m*128+t, j*128+p]
        xT = np.ascontiguousarray(
            xk.reshape(n_mt, 128, KCH, 128).transpose(3, 0, 2, 1)
              .reshape(128, n_mt * KCH * 128).astype(_BF16_NP))
        in_maps.append({"xT": xT, "wcT": wcT, "wwT": wwT, "msk": msk})

    LAST_RESULT = run_bass_kernel_spmd(
        nc, in_maps, list(range(NCORES)), trace=_trace,
        trace_cores=(_trace_cores if _trace else None))

    out = np.zeros((N, NCOL), np.float32)
    for k in range(NCORES):
        ok = np.asarray(LAST_RESULT.results[k]["out"], np.float32)
        ok[:, NCLS:] *= (1.0 / WSCALE)
        valid = tok_idx[k] >= 0
        out[tok_idx[k][valid]] = ok[valid]

    if overflow.size:
        # rare capacity-overflow tokens: evaluate directly on the host
        xo = x[overflow]                                   # [no, NHID]
        out[overflow, :NCLS] = xo @ Wc.T
        co = cls[overflow]
        out[overflow, NCLS:] = np.einsum(
            "nkh,nh->nk", Ww[co], xo, optimize=True)

    out[:, :NCLS] += bc
    out[:, NCLS:] += bw[cls]
    return out


# revision 5
# speedup vs baseline: 1.2745x; 1.1929x over previous
"""Class-based decoder (MoE-style routing) on 8 trn2 NeuronCores.

Strategy: expert-parallel. Classes are padded 250->256 and split 32 per core.
On the host, tokens are grouped by class into capacity-padded slots (C tokens
per class slot); the rare tokens beyond a class's capacity are evaluated on
the host.  Each core receives its padded tokens pre-transposed k-major (xT,
bf16) plus the replicated class-decoder weights and its word-decoder shard,
both quantized to float8_e3m4 pre-scaled by 64 (exact power of two, divided
back out on the host).  Per 128-token m-tile the PE computes class logits
(x @ Wc.T) and, per pair of classes, word logits (x @ [W_a|W_b].T) as K=512
accumulations with x as the stationary operand and the weights as the moving
operand, so every weight element streams through the PE exactly once.  Each
2C-row band's full pair block is copied out bf16 (one DVE cast per band) and
the host picks the diagonal during the unpermute — on-device selection costs
~420ns of fixed DVE overhead per extra instruction, which was the previous
bottleneck.

Why fp8: the 6.5 MB/core weight stream is the memory-bound bottleneck; e3m4
(4 mantissa bits) at scale 64 keeps all values normal and contributes ~1.2%
relative error (measured offline), under the 2e-2 gate, while x (stationary,
shared by both matmuls) stays bf16.
"""

import numpy as np
from contextlib import ExitStack

import concourse.bass as bass
import concourse.bacc as bacc
import concourse.tile as tile
import concourse.mybir as mybir
from concourse.bass_utils import run_bass_kernel_spmd

NHID = 512
NCLS = 250
CHUNK = 200
NCORES = 8
KCH = NHID // 128          # 4 contraction chunks of 128
NCLS_PAD = 256             # classes padded so each core owns an equal shard
CPC = NCLS_PAD // NCORES   # classes per core
NCOL = NCLS + CHUNK        # 450 output columns
F32 = mybir.dt.float32
BF16 = mybir.dt.bfloat16
F8 = mybir.dt.float8e3    # e3m4
WSCALE = 64.0              # weight pre-scale (power of two; divided out on host)
NWARM = 32                 # PE warm-up matmuls (HAM unthrottle + DMA ramp)

LAST_RESULT = None         # BassKernelResults of the most recent device run
_program_cache = {}

try:
    import ml_dtypes
    _BF16_NP = ml_dtypes.bfloat16
    _F8_NP = ml_dtypes.float8_e3m4
except ImportError:  # pragma: no cover - ml_dtypes ships with jax
    _BF16_NP = None
    _F8_NP = None


def _build_program(C, slots):
    """One SPMD program: slots class-slots of C tokens each, per core."""
    n_mt = (slots * C) // 128  # 128-token m-tiles
    npad = slots * C
    per_mt = 128 // C          # class slots per m-tile
    gs = 2 if per_mt >= 2 else 1
    gw = gs * CHUNK            # moving-operand width per word matmul
    n_half = per_mt // gs      # word matmul groups per m-tile
    hw = KCH * gw              # free-dim elems per half
    ncls_p = 256
    ocol = NCLS + gw           # wide rows: full pair block, host picks diag

    def wchunks(m):
        # W DMA chunks per m-tile as (first_half, n_halves): fine-grained on
        # the first m-tile (compute starts sooner) and the last (short tail)
        if n_half == 1 or m == 0 or m == n_mt - 1:
            return [(h, 1) for h in range(n_half)]
        return [(2 * q, 2) for q in range(n_half // 2)]

    nc = bacc.Bacc("TRN2", target_bir_lowering=False, debug=False,
                   num_devices=NCORES)
    xT = nc.dram_tensor("xT", [128, n_mt * KCH * 128], BF16,
                        kind="ExternalInput")
    wcT = nc.dram_tensor("wcT", [128, KCH * ncls_p], F8,
                         kind="ExternalInput")
    wwT = nc.dram_tensor("wwT", [n_mt, 128, n_half * hw], F8,
                         kind="ExternalInput")
    out = nc.dram_tensor("out", [npad, ocol], BF16, kind="ExternalOutput")

    with tile.TileContext(nc) as tc, ExitStack() as ctx:
        xpool = ctx.enter_context(tc.tile_pool(name="x", bufs=1))
        wcpool = ctx.enter_context(tc.tile_pool(name="wc", bufs=1))
        wpool = ctx.enter_context(tc.tile_pool(name="w", bufs=8))
        opool = ctx.enter_context(tc.tile_pool(name="o", bufs=4))
        wmpool = ctx.enter_context(tc.tile_pool(name="wm", bufs=1))
        pcp = ctx.enter_context(
            tc.tile_pool(name="pc", bufs=2, space=bass.MemorySpace.PSUM))
        pwp = ctx.enter_context(
            tc.tile_pool(name="pw", bufs=4, space=bass.MemorySpace.PSUM))
        pmp = ctx.enter_context(
            tc.tile_pool(name="pm", bufs=1, space=bass.MemorySpace.PSUM))

        # PE warm-up: HAM unthrottles only after ~3.4us of sustained PE
        # activity, and the input DMA ramp takes ~4us to deliver the first
        # real operands.  Burn that dead window with dummy matmuls so the
        # real ones start at full clock.
        warm_sb = wmpool.tile([128, 64], BF16)
        nc.vector.memset(warm_sb[:], 0.0)
        warm_ps = pmp.tile([64, 64], F32)
        for _ in range(NWARM):
            nc.tensor.matmul(warm_ps[:, :], warm_sb[:, :], warm_sb[:, :],
                             start=True, stop=True)

        # x first on the sync HWDGE queue (PE can't start without it), then
        # the W firehose in consumption order.  wc rides the scalar queue.
        x_sb = xpool.tile([128, n_mt * KCH * 128], BF16)
        nc.sync.dma_start(x_sb[:], xT[:])
        wc_sb = wcpool.tile([128, KCH * ncls_p], F8)
        nc.scalar.dma_start(wc_sb[:], wcT[:])
        w_sbs = []
        for m in range(n_mt):
            row = []
            for (h0, hn) in wchunks(m):
                w_sb = wpool.tile([128, hn * hw], F8, tag="w")
                nc.sync.dma_start(
                    w_sb[:], wwT[m][:, h0 * hw:(h0 + hn) * hw])
                row.append((h0, hn, w_sb))
            w_sbs.append(row)

        for m in range(n_mt):
            def xcol(j):
                base = (m * KCH + j) * 128
                return x_sb[:, base:base + 128]

            # class logits for these 128 tokens
            pc_ps = pcp.tile([128, ncls_p], F32)
            for j in range(KCH):
                nc.tensor.matmul(
                    pc_ps[:, :],
                    xcol(j),
                    wc_sb[:, j * ncls_p:(j + 1) * ncls_p],
                    start=(j == 0), stop=(j == KCH - 1),
                )
            o_sb = opool.tile([128, ocol], BF16)
            nc.scalar.copy(o_sb[:, :NCLS], pc_ps[:, :NCLS])

            # word logits: per half, one M=128 matmul of N=gw covering gs
            # classes; each gs*C-row band keeps its full pair block
            for (h0, hn, w_sb) in w_sbs[m]:
                for hh in range(hn):
                    h = h0 + hh
                    pw_ps = pwp.tile([128, gw], F32, tag="pw")
                    for j in range(KCH):
                        nc.tensor.matmul(
                            pw_ps[:, :],
                            xcol(j),
                            w_sb[:, (hh * KCH + j) * gw:
                                 (hh * KCH + j + 1) * gw],
                            start=(j == 0), stop=(j == KCH - 1),
                        )
                    b0, b1 = h * gs * C, (h + 1) * gs * C
                    nc.vector.tensor_copy(
                        o_sb[b0:b1, NCLS:], pw_ps[b0:b1, :])

            nc.scalar.dma_start(out[m * 128:(m + 1) * 128, :], o_sb[:])

    nc.compile()
    return nc


def _route(cls):
    """Group tokens by class into capacity-padded slots: one slot per class,
    C tokens of capacity.  The (rare) tokens beyond a class's capacity are
    returned as `overflow` and evaluated directly on the host in numpy.

    Returns (C, slots, tok_idx [NCORES, slots*C] int64 token id or -1,
    slot_cls [NCORES, slots] class id per slot, overflow token-id array).
    """
    counts = np.bincount(cls, minlength=NCLS_PAD)
    cands = (16, 32, 64, 128)
    C = cands[-1]
    for c in cands:
        if int(np.maximum(counts - c, 0).sum()) <= 32:
            C = c
            break

    order = np.argsort(cls, kind="stable")
    starts = np.zeros(NCLS_PAD + 1, np.int64)
    starts[1:] = np.cumsum(counts)

    slots = CPC  # one slot per class owned by the core
    tok_idx = np.full((NCORES, slots * C), -1, np.int64)
    slot_cls = np.full((NCORES, slots), -1, np.int64)
    overflow = []
    for k in range(NCORES):
        for s in range(slots):
            c = k * CPC + s
            lo, cnt = int(starts[c]), int(counts[c])
            n = min(C, cnt)
            slot_cls[k, s] = c
            if n > 0:
                tok_idx[k, s * C:s * C + n] = order[lo:lo + n]
            if cnt > C:
                overflow.append(order[lo + C:lo + cnt])
    overflow = (np.concatenate(overflow) if overflow
                else np.zeros((0,), np.int64))
    return C, slots, tok_idx, slot_cls, overflow


def kernel(x, Wc, bc, Ww, bw, cls_idx, _trace=False, _trace_cores=None):
    global LAST_RESULT

    x = np.ascontiguousarray(np.asarray(x, np.float32))
    Wc = np.ascontiguousarray(np.asarray(Wc, np.float32))
    bc = np.asarray(bc, np.float32)
    Ww = np.ascontiguousarray(np.asarray(Ww, np.float32))
    bw = np.asarray(bw, np.float32)
    cls = np.asarray(cls_idx).astype(np.int64).ravel()
    N = cls.shape[0]

    C, slots, tok_idx, slot_cls, overflow = _route(cls)
    npad = slots * C
    n_mt = npad // 128
    per_mt = 128 // C
    gs = 2 if per_mt >= 2 else 1
    ncls_p = 256

    key = (C, slots)
    if key not in _program_cache:
        _program_cache[key] = _build_program(C, slots)
    nc = _program_cache[key]

    # wcT [128, KCH*ncls_p]: wcT[p, j*ncls_p+c] = Wc[c, j*128+p]  (replicated)
    Wc_p = np.concatenate(
        [Wc, np.zeros((ncls_p - NCLS, NHID), np.float32)], 0)
    wcT = np.ascontiguousarray(
        (Wc_p.reshape(ncls_p, KCH, 128).transpose(2, 1, 0)
         .reshape(128, KCH * ncls_p) * WSCALE).astype(_F8_NP))

    Ww_pad = np.zeros((NCLS_PAD, CHUNK, NHID), np.float32)
    Ww_pad[:NCLS] = Ww

    in_maps = []
    for k in range(NCORES):
        # per-slot k-major weights: tmp[s, j, p, w] = Ww[cls_s, w, j*128+p]
        wsel = Ww_pad[np.maximum(slot_cls[k], 0)]
        wsel[slot_cls[k] < 0] = 0.0
        tmp = wsel.reshape(slots, CHUNK, KCH, 128).transpose(0, 2, 3, 1)
        if gs == 2:
            # group = m-tile (per_mt slots); within: pair h, then j, then
            # the two slots' CHUNK columns side by side
            tmp = tmp.reshape(n_mt, per_mt // 2, 2, KCH, 128, CHUNK)
            tmp = tmp.transpose(0, 4, 1, 3, 2, 5)  # [n_mt,128,pair,j,2,CHUNK]
        else:
            tmp = tmp.reshape(n_mt, per_mt, KCH, 128, CHUNK)
            tmp = tmp.transpose(0, 3, 1, 2, 4)     # [n_mt,128,q,j,CHUNK]
        wwT = np.ascontiguousarray(
            (tmp.reshape(n_mt, 128, per_mt * KCH * CHUNK) * WSCALE)
            .astype(_F8_NP))

        ti = tok_idx[k]
        xk = x[np.maximum(ti, 0)]
        xk[ti < 0] = 0.0
        # xT[p, (m*KCH+j)*128 + t] = xk[m*128+t, j*128+p]
        xT = np.ascontiguousarray(
            xk.reshape(n_mt, 128, KCH, 128).transpose(3, 0, 2, 1)
              .reshape(128, n_mt * KCH * 128).astype(_BF16_NP))
        in_maps.append({"xT": xT, "wcT": wcT, "wwT": wwT})

    LAST_RESULT = run_bass_kernel_spmd(
        nc, in_maps, list(range(NCORES)), trace=_trace,
        trace_cores=(_trace_cores if _trace else None))

    out = np.zeros((N, NCOL), np.float32)
    if gs == 2:
        # row r of a core's output holds its pair's full 2*CHUNK block;
        # slot parity selects which CHUNK half is this row's class
        a_row = (np.arange(npad) // C) % 2
    for k in range(NCORES):
        ok = np.asarray(LAST_RESULT.results[k]["out"], np.float32)
        if gs == 2:
            words = np.where((a_row == 0)[:, None],
                             ok[:, NCLS:NCLS + CHUNK],
                             ok[:, NCLS + CHUNK:NCLS + 2 * CHUNK])
            ok = np.concatenate([ok[:, :NCLS], words], 1)
        ok *= (1.0 / WSCALE)
        valid = tok_idx[k] >= 0
        out[tok_idx[k][valid]] = ok[valid]

    if overflow.size:
        # rare capacity-overflow tokens: evaluate directly on the host
        xo = x[overflow]                                   # [no, NHID]
        out[overflow, :NCLS] = xo @ Wc.T
        co = cls[overflow]
        out[overflow, NCLS:] = np.einsum(
            "nkh,nh->nk", Ww[co], xo, optimize=True)

    out[:, :NCLS] += bc
    out[:, NCLS:] += bw[cls]
    return out


# revision 6
# speedup vs baseline: 1.3164x; 1.0329x over previous
"""Class-based decoder (MoE-style routing) on 8 trn2 NeuronCores.

Strategy: expert-parallel. Classes are padded 250->256 and split 32 per core.
On the host, tokens are grouped by class into capacity-padded slots (C tokens
per class slot); the rare tokens beyond a class's capacity are evaluated on
the host.  Each core receives its padded tokens pre-transposed k-major (xT,
bf16) plus the replicated class-decoder weights and its word-decoder shard,
both quantized to float8_e3m4 pre-scaled by 64 (exact power of two, divided
back out on the host).  Per 128-token m-tile the PE computes class logits
(x @ Wc.T) and, per pair of classes, word logits (x @ [W_a|W_b].T) as K=512
accumulations with x as the stationary operand and the weights as the moving
operand, so every weight element streams through the PE exactly once.  Each
2C-row band's full pair block is copied out bf16 (one DVE cast per band) and
the host picks the diagonal during the unpermute — on-device selection costs
~420ns of fixed DVE overhead per extra instruction, which was the previous
bottleneck.

Why fp8: the 6.5 MB/core weight stream is the memory-bound bottleneck; e3m4
(4 mantissa bits) at scale 64 keeps all values normal and contributes ~1.2%
relative error (measured offline), under the 2e-2 gate, while x (stationary,
shared by both matmuls) stays bf16.
"""

import numpy as np
from contextlib import ExitStack

import concourse.bass as bass
import concourse.bacc as bacc
import concourse.tile as tile
import concourse.mybir as mybir
from concourse.bass_utils import run_bass_kernel_spmd

NHID = 512
NCLS = 250
CHUNK = 200
NCORES = 8
KCH = NHID // 128          # 4 contraction chunks of 128
NCLS_PAD = 256             # classes padded so each core owns an equal shard
CPC = NCLS_PAD // NCORES   # classes per core
NCOL = NCLS + CHUNK        # 450 output columns
F32 = mybir.dt.float32
BF16 = mybir.dt.bfloat16
F8 = mybir.dt.float8e3    # e3m4
WSCALE = 64.0              # weight pre-scale (power of two; divided out on host)
NWARM = 32                 # PE warm-up matmuls (HAM unthrottle + DMA ramp)

LAST_RESULT = None         # BassKernelResults of the most recent device run
_program_cache = {}

try:
    import ml_dtypes
    _BF16_NP = ml_dtypes.bfloat16
    _F8_NP = ml_dtypes.float8_e3m4
except ImportError:  # pragma: no cover - ml_dtypes ships with jax
    _BF16_NP = None
    _F8_NP = None


def _build_program(C, slots):
    """One SPMD program: slots class-slots of C tokens each, per core."""
    n_mt = (slots * C) // 128  # 128-token m-tiles
    npad = slots * C
    per_mt = 128 // C          # class slots per m-tile
    gs = 2 if per_mt >= 2 else 1
    gw = gs * CHUNK            # moving-operand width per word matmul
    n_half = per_mt // gs      # word matmul groups per m-tile
    hw = KCH * gw              # free-dim elems per half
    ncls_p = 256
    ocol = NCLS + gw           # wide rows: full pair block, host picks diag

    def wchunks(m):
        # W DMA chunks per m-tile as (first_half, n_halves): fine-grained on
        # the first m-tile (compute starts sooner) and the last (short tail)
        if n_half == 1 or m == 0 or m == n_mt - 1:
            return [(h, 1) for h in range(n_half)]
        return [(2 * q, 2) for q in range(n_half // 2)]

    nc = bacc.Bacc("TRN2", target_bir_lowering=False, debug=False,
                   num_devices=NCORES)
    xT = nc.dram_tensor("xT", [128, n_mt * KCH * 128], BF16,
                        kind="ExternalInput")
    wcT = nc.dram_tensor("wcT", [128, KCH * ncls_p], F8,
                         kind="ExternalInput")
    wwT = nc.dram_tensor("wwT", [n_mt, 128, n_half * hw], F8,
                         kind="ExternalInput")
    out = nc.dram_tensor("out", [npad, ocol], BF16, kind="ExternalOutput")

    with tile.TileContext(nc) as tc, ExitStack() as ctx:
        xpool = ctx.enter_context(tc.tile_pool(name="x", bufs=1))
        wcpool = ctx.enter_context(tc.tile_pool(name="wc", bufs=1))
        wpool = ctx.enter_context(tc.tile_pool(name="w", bufs=8))
        opool = ctx.enter_context(tc.tile_pool(name="o", bufs=3))
        wmpool = ctx.enter_context(tc.tile_pool(name="wm", bufs=1))
        pcp = ctx.enter_context(
            tc.tile_pool(name="pc", bufs=2, space=bass.MemorySpace.PSUM))
        pwp = ctx.enter_context(
            tc.tile_pool(name="pw", bufs=5, space=bass.MemorySpace.PSUM))

        # PE warm-up: HAM unthrottles only after ~3.4us of sustained PE
        # activity, and the input DMA ramp takes ~3us to deliver the first
        # real operands.  Burn that dead window with dummy matmuls so the
        # real ones start at full clock.
        warm_sb = wmpool.tile([128, 64], BF16)
        nc.vector.memset(warm_sb[:], 0.0)
        warm_ps = pcp.tile([64, 64], F32, tag="warm", bufs=1)
        for _ in range(NWARM):
            nc.tensor.matmul(warm_ps[:, :], warm_sb[:, :], warm_sb[:, :],
                             start=True, stop=True)

        # One sync-HWDGE queue in exact consumption order: wc, then per
        # m-tile its x slice followed by its W chunks.  PE tracks the stream
        # with ~0.5us lag instead of waiting for one big x load.
        wc_sb = wcpool.tile([128, KCH * ncls_p], F8)
        nc.sync.dma_start(wc_sb[:], wcT[:])
        x_sb = xpool.tile([128, n_mt * KCH * 128], BF16)
        w_sbs = []
        for m in range(n_mt):
            nc.sync.dma_start(
                x_sb[:, m * KCH * 128:(m + 1) * KCH * 128],
                xT[:, m * KCH * 128:(m + 1) * KCH * 128])
            row = []
            for (h0, hn) in wchunks(m):
                w_sb = wpool.tile([128, hn * hw], F8, tag="w")
                nc.sync.dma_start(
                    w_sb[:], wwT[m][:, h0 * hw:(h0 + hn) * hw])
                row.append((h0, hn, w_sb))
            w_sbs.append(row)

        for m in range(n_mt):
            def xcol(j):
                base = (m * KCH + j) * 128
                return x_sb[:, base:base + 128]

            # class logits for these 128 tokens
            pc_ps = pcp.tile([128, ncls_p], F32, tag="pc")
            for j in range(KCH):
                nc.tensor.matmul(
                    pc_ps[:, :],
                    xcol(j),
                    wc_sb[:, j * ncls_p:(j + 1) * ncls_p],
                    start=(j == 0), stop=(j == KCH - 1),
                )
            o_sb = opool.tile([128, ocol], BF16)
            nc.scalar.copy(o_sb[:, :NCLS], pc_ps[:, :NCLS])

            # word logits: per half, one M=128 matmul of N=gw covering gs
            # classes; each gs*C-row band keeps its full pair block
            for (h0, hn, w_sb) in w_sbs[m]:
                for hh in range(hn):
                    h = h0 + hh
                    pw_ps = pwp.tile([128, gw], F32, tag="pw")
                    for j in range(KCH):
                        nc.tensor.matmul(
                            pw_ps[:, :],
                            xcol(j),
                            w_sb[:, (hh * KCH + j) * gw:
                                 (hh * KCH + j + 1) * gw],
                            start=(j == 0), stop=(j == KCH - 1),
                        )
                    b0, b1 = h * gs * C, (h + 1) * gs * C
                    nc.vector.tensor_copy(
                        o_sb[b0:b1, NCLS:], pw_ps[b0:b1, :])
                    # store each finished 64-row half so the last store
                    # after the final W chunk is small
                    if n_half >= 2 and b1 in (64, 128):
                        r0 = b1 - 64
                        nc.scalar.dma_start(
                            out[m * 128 + r0:m * 128 + b1, :],
                            o_sb[r0:b1, :])

            if n_half < 2:
                nc.scalar.dma_start(out[m * 128:(m + 1) * 128, :], o_sb[:])

    nc.compile()
    return nc


def _route(cls):
    """Group tokens by class into capacity-padded slots: one slot per class,
    C tokens of capacity.  The (rare) tokens beyond a class's capacity are
    returned as `overflow` and evaluated directly on the host in numpy.

    Returns (C, slots, tok_idx [NCORES, slots*C] int64 token id or -1,
    slot_cls [NCORES, slots] class id per slot, overflow token-id array).
    """
    counts = np.bincount(cls, minlength=NCLS_PAD)
    cands = (16, 32, 64, 128)
    C = cands[-1]
    for c in cands:
        if int(np.maximum(counts - c, 0).sum()) <= 32:
            C = c
            break

    order = np.argsort(cls, kind="stable")
    starts = np.zeros(NCLS_PAD + 1, np.int64)
    starts[1:] = np.cumsum(counts)

    slots = CPC  # one slot per class owned by the core
    tok_idx = np.full((NCORES, slots * C), -1, np.int64)
    slot_cls = np.full((NCORES, slots), -1, np.int64)
    overflow = []
    for k in range(NCORES):
        for s in range(slots):
            c = k * CPC + s
            lo, cnt = int(starts[c]), int(counts[c])
            n = min(C, cnt)
            slot_cls[k, s] = c
            if n > 0:
                tok_idx[k, s * C:s * C + n] = order[lo:lo + n]
            if cnt > C:
                overflow.append(order[lo + C:lo + cnt])
    overflow = (np.concatenate(overflow) if overflow
                else np.zeros((0,), np.int64))
    return C, slots, tok_idx, slot_cls, overflow


def kernel(x, Wc, bc, Ww, bw, cls_idx, _trace=False, _trace_cores=None):
    global LAST_RESULT

    x = np.ascontiguousarray(np.asarray(x, np.float32))
    Wc = np.ascontiguousarray(np.asarray(Wc, np.float32))
    bc = np.asarray(bc, np.float32)
    Ww = np.ascontiguousarray(np.asarray(Ww, np.float32))
    bw = np.asarray(bw, np.float32)
    cls = np.asarray(cls_idx).astype(np.int64).ravel()
    N = cls.shape[0]

    C, slots, tok_idx, slot_cls, overflow = _route(cls)
    npad = slots * C
    n_mt = npad // 128
    per_mt = 128 // C
    gs = 2 if per_mt >= 2 else 1
    ncls_p = 256

    key = (C, slots)
    if key not in _program_cache:
        _program_cache[key] = _build_program(C, slots)
    nc = _program_cache[key]

    # wcT [128, KCH*ncls_p]: wcT[p, j*ncls_p+c] = Wc[c, j*128+p]  (replicated)
    Wc_p = np.concatenate(
        [Wc, np.zeros((ncls_p - NCLS, NHID), np.float32)], 0)
    wcT = np.ascontiguousarray(
        (Wc_p.reshape(ncls_p, KCH, 128).transpose(2, 1, 0)
         .reshape(128, KCH * ncls_p) * WSCALE).astype(_F8_NP))

    Ww_pad = np.zeros((NCLS_PAD, CHUNK, NHID), np.float32)
    Ww_pad[:NCLS] = Ww

    in_maps = []
    for k in range(NCORES):
        # per-slot k-major weights: tmp[s, j, p, w] = Ww[cls_s, w, j*128+p]
        wsel = Ww_pad[np.maximum(slot_cls[k], 0)]
        wsel[slot_cls[k] < 0] = 0.0
        tmp = wsel.reshape(slots, CHUNK, KCH, 128).transpose(0, 2, 3, 1)
        if gs == 2:
            # group = m-tile (per_mt slots); within: pair h, then j, then
            # the two slots' CHUNK columns side by side
            tmp = tmp.reshape(n_mt, per_mt // 2, 2, KCH, 128, CHUNK)
            tmp = tmp.transpose(0, 4, 1, 3, 2, 5)  # [n_mt,128,pair,j,2,CHUNK]
        else:
            tmp = tmp.reshape(n_mt, per_mt, KCH, 128, CHUNK)
            tmp = tmp.transpose(0, 3, 1, 2, 4)     # [n_mt,128,q,j,CHUNK]
        wwT = np.ascontiguousarray(
            (tmp.reshape(n_mt, 128, per_mt * KCH * CHUNK) * WSCALE)
            .astype(_F8_NP))

        ti = tok_idx[k]
        xk = x[np.maximum(ti, 0)]
        xk[ti < 0] = 0.0
        # xT[p, (m*KCH+j)*128 + t] = xk[m*128+t, j*128+p]
        xT = np.ascontiguousarray(
            xk.reshape(n_mt, 128, KCH, 128).transpose(3, 0, 2, 1)
              .reshape(128, n_mt * KCH * 128).astype(_BF16_NP))
        in_maps.append({"xT": xT, "wcT": wcT, "wwT": wwT})

    LAST_RESULT = run_bass_kernel_spmd(
        nc, in_maps, list(range(NCORES)), trace=_trace,
        trace_cores=(_trace_cores if _trace else None))

    out = np.zeros((N, NCOL), np.float32)
    if gs == 2:
        # row r of a core's output holds its pair's full 2*CHUNK block;
        # slot parity selects which CHUNK half is this row's class
        a_row = (np.arange(npad) // C) % 2
    for k in range(NCORES):
        ok = np.asarray(LAST_RESULT.results[k]["out"], np.float32)
        if gs == 2:
            words = np.where((a_row == 0)[:, None],
                             ok[:, NCLS:NCLS + CHUNK],
                             ok[:, NCLS + CHUNK:NCLS + 2 * CHUNK])
            ok = np.concatenate([ok[:, :NCLS], words], 1)
        ok *= (1.0 / WSCALE)
        valid = tok_idx[k] >= 0
        out[tok_idx[k][valid]] = ok[valid]

    if overflow.size:
        # rare capacity-overflow tokens: evaluate directly on the host
        xo = x[overflow]                                   # [no, NHID]
        out[overflow, :NCLS] = xo @ Wc.T
        co = cls[overflow]
        out[overflow, NCLS:] = np.einsum(
            "nkh,nh->nk", Ww[co], xo, optimize=True)

    out[:, :NCLS] += bc
    out[:, NCLS:] += bw[cls]
    return out
